# revision 21
# baseline (speedup 1.0000x reference)
"""Trainium2 Bass kernel for nn_Decoder (2-layer-norm decoder block).

Sharding: 8 cores = 2 batches x 4 query-chunks of 512 rows. Zero collectives:
each core computes K/V for its whole batch (redundant across the 4 cores of a
batch, SPMD-uniform) and produces its own 512 output rows end-to-end.

Layout: activations live transposed in SBUF as [embed(part), rows(free)] so
every matmul is a natural lhsT.T @ rhs with no on-chip transposes.  Attention
scores are computed transposed, scoresT[k, q] = KT_h.T @ QT_h; softmax sums
ride the attention-value matmul as an extra ones-column in V; the causal mask
is a per-core bf16 input multiplied into P after exp.  All fp32 matmuls are
issued as float32r (full PE rate at moving dim >= 256); P/V of the attention
value matmul are bf16.
"""

import numpy as np
import ml_dtypes

import concourse.bass as bass
import concourse.bacc as bacc
import concourse.mybir as mybir
import concourse.tile as tile
from concourse import bass_utils
from concourse.bass_interp import get_hw_module

F32 = mybir.dt.float32
F32R = mybir.dt.float32r
BF16 = mybir.dt.bfloat16
AF = mybir.ActivationFunctionType
OP = mybir.AluOpType

E = 512          # embed
NH = 8           # heads
DH = 64          # head dim
N = 2048         # sequence length
B = 2            # batch
CH = 512         # query chunk per core
NCORES = 8
HID = 2048       # FFN hidden
EPS = 1e-5
ET = E // 128    # 4 embed tiles
KT_N = N // 128  # 16 key tiles
NG = KT_N // 2   # 8 score groups (2 key tiles each)

# order of [512] vectors packed into the "vecs" [128, 60] input
VEC_NAMES = [
    "sa_bq", "sa_bk", "sa_bv", "sa_bo",
    "ca_bq", "ca_bk", "ca_bv", "ca_bo",
    "ln1_g", "ln1_b", "ln2_g", "ln2_b", "ln3_g", "ln3_b",
    "ff_b2",
]


def build_program(debug=False):
    nc = bacc.Bacc("TRN2", target_bir_lowering=False, debug=False,
                   enable_asserts=False, num_devices=NCORES)

    # ---- DRAM I/O (per-core contents differ; program is SPMD-uniform) ----
    d_xq = nc.dram_tensor("xqT", [E, CH], F32R, kind="ExternalInput").ap()
    d_xb = nc.dram_tensor("xbT", [E, N], F32R, kind="ExternalInput").ap()
    d_enc = nc.dram_tensor("encT", [E, N], F32R, kind="ExternalInput").ap()
    d_mask = nc.dram_tensor("maskg", [NG, 128, 1024], BF16, kind="ExternalInput").ap()
    d_w = {}
    for p in ("sa", "ca"):
        for w in ("wq", "wk", "wv", "wo"):
            d_w[f"{p}_{w}"] = nc.dram_tensor(f"{p}_{w}", [E, E], F32R,
                                             kind="ExternalInput").ap()
    d_w["ff_w1"] = nc.dram_tensor("ff_w1", [E, HID], F32R, kind="ExternalInput").ap()
    d_w["ff_w2"] = nc.dram_tensor("ff_w2", [HID, E], BF16, kind="ExternalInput").ap()
    d_vecs = nc.dram_tensor("vecs", [128, 4 * len(VEC_NAMES)], F32,
                            kind="ExternalInput").ap()
    d_b1 = nc.dram_tensor("b1p", [128, HID // 128], F32, kind="ExternalInput").ap()
    d_bvrows = nc.dram_tensor("bvrows", [1, 2 * E], F32, kind="ExternalInput").ap()
    d_onesid = nc.dram_tensor("onesid", [128, 129], F32R, kind="ExternalInput").ap()
    d_out = nc.dram_tensor("out", [CH, E], F32R, kind="ExternalOutput").ap()
    d_dbg = {}
    if debug:
        for nm, sh, dt_ in (("KT", [128, ET * N], BF16), ("V", [128, KT_N * NH * 65], BF16),
                            ("QT", [128, ET * CH], BF16), ("av1", [128, ET * CH], F32R),
                            ("A1", [128, ET * CH], F32R), ("B1", [128, ET * CH], F32R),
                            ("x2T", [128, ET * CH], F32R), ("hT", [128, ET * N], BF16)):
            d_dbg[nm] = nc.dram_tensor("dbg_" + nm, sh, dt_, kind="ExternalOutput").ap()

    with tile.TileContext(nc) as tc:
        _build(nc, tc, d_xq, d_xb, d_enc, d_mask, d_w, d_vecs, d_b1, d_bvrows, d_onesid, d_out,
               d_dbg)
    nc.compile()
    return nc


def _build(nc, tc, d_xq, d_xb, d_enc, d_mask, d_w, d_vecs, d_b1, d_bvrows, d_onesid, d_out,
           d_dbg=None):
    from contextlib import ExitStack
    ctx = ExitStack()
    per = ctx.enter_context(tc.tile_pool(name="per", bufs=1))
    wpool = ctx.enter_context(tc.tile_pool(name="w", bufs=8))
    w2pool = ctx.enter_context(tc.tile_pool(name="w2", bufs=16))
    xbp = ctx.enter_context(tc.tile_pool(name="xb", bufs=4))
    ppool = ctx.enter_context(tc.tile_pool(name="p", bufs=2))
    tmp = ctx.enter_context(tc.tile_pool(name="tmp", bufs=2))
    stat = ctx.enter_context(tc.tile_pool(name="stat", bufs=2))
    lnstat = ctx.enter_context(tc.tile_pool(name="lnstat", bufs=1))
    bc = ctx.enter_context(tc.tile_pool(name="bc", bufs=2))
    bcl = ctx.enter_context(tc.tile_pool(name="bcl", bufs=1))
    ps_sc = ctx.enter_context(tc.tile_pool(name="ps_sc", bufs=2, space="PSUM"))
    ps_av = ctx.enter_context(tc.tile_pool(name="ps_av", bufs=2, space="PSUM"))
    ps_pp = ctx.enter_context(tc.tile_pool(name="ps_pp", bufs=2, space="PSUM"))

    # ---- persistent SBUF tensors ----
    xq = per.tile([128, ET * CH], F32R, tag="xq")          # e-tile blocks of xqT
    KT = per.tile([128, ET * N], BF16, tag="KT")          # [d, k]; reused as hT in FFN
    V = per.tile([128, KT_N * (NH * 65)], BF16, tag="V")  # [k, h*65] (64 d + ones col)
    QT = per.tile([128, ET * CH], BF16, tag="QT")
    mask = per.tile([128, NG * 1024], BF16, tag="mask")
    av = per.tile([128, ET * CH], F32R, tag="av")          # [d, q]; reused as out_sb
    A = per.tile([128, ET * CH], F32R, tag="A")            # x1pre / x2pre / x3pre
    Bt = per.tile([128, ET * CH], F32R, tag="B")           # x1T / x3T
    x2T = per.tile([128, ET * CH], F32R, tag="x2T")
    vecs = per.tile([128, 4 * len(VEC_NAMES)], F32, tag="vecs")
    b1p = per.tile([128, HID // 128], F32, tag="b1p")
    bvrows = per.tile([1, 2 * E], F32, tag="bvrows")
    onesid = per.tile([128, 129], F32R, tag="onesid")
    ones = onesid[:, 0:1]
    ident = onesid[:, 1:129]
    epst = per.tile([1, 1], F32, tag="epst")

    def vcol(name, dt):
        v = VEC_NAMES.index(name)
        return vecs[:, 4 * v + dt: 4 * v + dt + 1]

    dma = nc.sync.dma_start

    # ---- loads ----
    for et in range(ET):
        dma(xq[:, CH * et: CH * (et + 1)], d_xq[128 * et: 128 * (et + 1), :])
    for g in range(NG):
        dma(mask[:, 1024 * g: 1024 * (g + 1)], d_mask[g])
    dma(vecs[:], d_vecs)
    dma(b1p[:], d_b1)
    dma(bvrows[:], d_bvrows)
    dma(onesid[:], d_onesid)
    nc.vector.memset(epst[:], EPS)

    def load_w(name, cols=E):
        ts = []
        for et in range(ET):
            t = wpool.tile([128, cols], F32R, tag="w")
            dma(t[:], d_w[name][128 * et: 128 * (et + 1), :])
            ts.append(t)
        return ts

    def kv_proj(src_tiles, wk_name, bk_name, wv_name, bv_slice):
        """K/V projections over the full batch (keys)."""
        wk = load_w(wk_name)
        wv = load_w(wv_name)
        # KT[d, k] = wk.T @ xT
        for dt in range(ET):
            for kb in range(N // 512):
                ps = ps_pp.tile([128, 512], F32, tag="pp")
                for eb in range(ET):
                    nc.tensor.matmul(
                        ps[:], wk[eb][:, 128 * dt: 128 * (dt + 1)],
                        src_tiles[eb][:, 512 * kb: 512 * (kb + 1)],
                        start=(eb == 0), stop=(eb == ET - 1))
                nc.vector.tensor_scalar(
                    KT[:, N * dt + 512 * kb: N * dt + 512 * (kb + 1)],
                    ps[:], vcol(bk_name, dt), None, OP.add)
        # V[k, d] = xT_slice.T @ wv   (+ ones col per head)
        bvb = bc.tile([128, 512], F32, tag="bvb")
        nc.gpsimd.partition_broadcast(bvb[:], bv_slice)
        for kt in range(KT_N):
            ps = ps_pp.tile([128, 512], F32, tag="pp")
            eb_src = kt // 4
            for eb in range(ET):
                nc.tensor.matmul(
                    ps[:], src_tiles[eb][:, 128 * kt: 128 * (kt + 1)],
                    wv[eb], start=(eb == 0), stop=(eb == ET - 1))
            vt = V[:, (NH * 65) * kt: (NH * 65) * (kt + 1)]
            vt3 = vt.rearrange("p (h d) -> p h d", h=NH)
            nc.vector.tensor_add(
                vt3[:, :, 0:64], ps[:].rearrange("p (h d) -> p h d", h=NH),
                bvb[:].rearrange("p (h d) -> p h d", h=NH))
            nc.vector.memset(vt3[:, :, 64:65], 1.0)
        del eb_src

    def q_proj(src, wq_name, bq_name):
        wq = load_w(wq_name)
        for dt in range(ET):
            ps = ps_pp.tile([128, 512], F32, tag="pp")
            for eb in range(ET):
                nc.tensor.matmul(
                    ps[:], wq[eb][:, 128 * dt: 128 * (dt + 1)],
                    src[:, CH * eb: CH * (eb + 1)],
                    start=(eb == 0), stop=(eb == ET - 1))
            nc.vector.tensor_scalar(
                QT[:, CH * dt: CH * (dt + 1)], ps[:], vcol(bq_name, dt), None, OP.add)

    def attention(masked):
        """QT/KT/V -> av (normalized attention values, [d, q] layout)."""
        for h in range(NH):
            dt, r = h // 2, h % 2
            avp = ps_av.tile([65, 512], F32, tag="av")
            for g in range(NG):
                sc = ps_sc.tile([128, 1024], F32, tag="sc")
                for j in range(2):
                    kt = 2 * g + j
                    nc.tensor.matmul(
                        sc[:, 512 * j: 512 * (j + 1)],
                        KT[64 * r: 64 * (r + 1), N * dt + 128 * kt: N * dt + 128 * (kt + 1)],
                        QT[64 * r: 64 * (r + 1), CH * dt: CH * (dt + 1)],
                        start=True, stop=True)
                p = ppool.tile([128, 1024], BF16, tag="p")
                nc.scalar.activation(p[:], sc[:], AF.Exp, scale=0.125)
                if masked:
                    nc.vector.tensor_mul(p[:], p[:], mask[:, 1024 * g: 1024 * (g + 1)])
                for j in range(2):
                    kt = 2 * g + j
                    nc.tensor.matmul(
                        avp[:], V[:, (NH * 65) * kt + 65 * h: (NH * 65) * kt + 65 * (h + 1)],
                        p[:, 512 * j: 512 * (j + 1)],
                        start=(kt == 0), stop=(kt == KT_N - 1))
            # normalize by the ones-column sums (row 64) and store to av.
            # NB: partition_broadcast on HW reads the tile's absolute partition
            # 0, so the sums row must be copied down to partition 0 first.
            srow = stat.tile([1, 512], F32, tag="srow")
            rrow = stat.tile([1, 512], F32, tag="rrow")
            nc.vector.tensor_copy(srow[:], avp[64:65, :])
            nc.vector.reciprocal(rrow[:], srow[:])
            rb = bc.tile([64, 512], F32, tag="rb")
            nc.gpsimd.partition_broadcast(rb[:], rrow[:])
            nc.vector.tensor_mul(
                av[64 * r: 64 * (r + 1), CH * dt: CH * (dt + 1)], avp[0:64, :], rb[:])

    def o_proj(wo_name, bo_name, resid, dst):
        wo = load_w(wo_name)
        for et in range(ET):
            ps = ps_pp.tile([128, 512], F32, tag="pp")
            for dt in range(ET):
                nc.tensor.matmul(
                    ps[:], wo[dt][:, 128 * et: 128 * (et + 1)],
                    av[:, CH * dt: CH * (dt + 1)],
                    start=(dt == 0), stop=(dt == ET - 1))
            nc.vector.scalar_tensor_tensor(
                dst[:, CH * et: CH * (et + 1)], ps[:], vcol(bo_name, et),
                resid[:, CH * et: CH * (et + 1)], OP.add, OP.add)

    def layernorm(src, dst, g_name, b_name):
        sums = ps_pp.tile([1, 512], F32, tag="pp")
        sumsq = ps_pp.tile([1, 512], F32, tag="pp")
        for et in range(ET):
            sq = tmp.tile([128, 512], F32R, tag="sq")
            nc.scalar.activation(sq[:], src[:, CH * et: CH * (et + 1)], AF.Square)
            nc.tensor.matmul(sums[:], ones, src[:, CH * et: CH * (et + 1)],
                             start=(et == 0), stop=(et == ET - 1))
            nc.tensor.matmul(sumsq[:], ones, sq[:],
                             start=(et == 0), stop=(et == ET - 1))
        mu = lnstat.tile([1, 512], F32, tag="mu")
        var = lnstat.tile([1, 512], F32, tag="var")
        rinv = lnstat.tile([1, 512], F32, tag="rinv")
        nc.vector.tensor_scalar_mul(mu[:], sums[:], 1.0 / E)
        # var = sumsq/E - mu^2  (computed as (sumsq/E) + (-mu*mu))
        nc.vector.tensor_scalar_mul(var[:], sumsq[:], 1.0 / E)
        musq = lnstat.tile([1, 512], F32, tag="musq")
        nc.scalar.activation(musq[:], mu[:], AF.Square)
        nc.vector.tensor_sub(var[:], var[:], musq[:])
        nc.scalar.activation(var[:], var[:], AF.Sqrt, bias=epst[:])
        nc.vector.reciprocal(rinv[:], var[:])
        mub = bcl.tile([128, 512], F32, tag="mub")
        rb = bcl.tile([128, 512], F32, tag="rstdb")
        nc.gpsimd.partition_broadcast(mub[:], mu[:])
        nc.gpsimd.partition_broadcast(rb[:], rinv[:])
        for et in range(ET):
            t = tmp.tile([128, 512], F32R, tag="sq")
            nc.vector.tensor_sub(t[:], src[:, CH * et: CH * (et + 1)], mub[:])
            nc.vector.tensor_mul(t[:], t[:], rb[:])
            nc.vector.tensor_scalar(
                dst[:, CH * et: CH * (et + 1)], t[:],
                vcol(g_name, et), vcol(b_name, et), OP.mult, OP.add)

    # ================= phase 1: self-attention =================
    xb = []
    for eb in range(ET):
        t = xbp.tile([128, N], F32R, tag="xb")
        dma(t[:], d_xb[128 * eb: 128 * (eb + 1), :])
        xb.append(t)
    kv_proj(xb, "sa_wk", "sa_bk", "sa_wv", bvrows[0:1, 0:E])
    q_proj(xq, "sa_wq", "sa_bq")
    if d_dbg:
        dma(d_dbg["KT"], KT[:])
        dma(d_dbg["V"], V[:])
        dma(d_dbg["QT"], QT[:])
    attention(masked=True)
    if d_dbg:
        dma(d_dbg["av1"], av[:])
    o_proj("sa_wo", "sa_bo", xq, A)          # A = attn + x
    if d_dbg:
        dma(d_dbg["A1"], A[:])
    layernorm(A, Bt, "ln1_g", "ln1_b")       # Bt = x1T
    if d_dbg:
        dma(d_dbg["B1"], Bt[:])

    # ================= phase 2: cross-attention =================
    enc = []
    for eb in range(ET):
        t = xbp.tile([128, N], F32R, tag="xb")
        dma(t[:], d_enc[128 * eb: 128 * (eb + 1), :])
        enc.append(t)
    kv_proj(enc, "ca_wk", "ca_bk", "ca_wv", bvrows[0:1, E:2 * E])
    q_proj(Bt, "ca_wq", "ca_bq")
    attention(masked=False)
    o_proj("ca_wo", "ca_bo", Bt, A)          # A = x2pre
    layernorm(A, x2T, "ln2_g", "ln2_b")
    if d_dbg:
        dma(d_dbg["x2T"], x2T[:])

    # ================= phase 3: FFN =================
    w1 = []
    for eb in range(ET):
        t = xbp.tile([128, HID], F32R, tag="xb")
        dma(t[:], d_w["ff_w1"][128 * eb: 128 * (eb + 1), :])
        w1.append(t)
    for ht in range(HID // 128):
        ps = ps_pp.tile([128, 512], F32, tag="pp")
        for eb in range(ET):
            nc.tensor.matmul(
                ps[:], w1[eb][:, 128 * ht: 128 * (ht + 1)],
                x2T[:, CH * eb: CH * (eb + 1)],
                start=(eb == 0), stop=(eb == ET - 1))
        # hT reuses KT's slots ([128, 8192])
        nc.scalar.activation(KT[:, 512 * ht: 512 * (ht + 1)], ps[:], AF.Gelu,
                             bias=b1p[:, ht: ht + 1])
    w2 = []
    for ht in range(HID // 128):
        t = w2pool.tile([128, 512], BF16, tag="w2")
        dma(t[:], d_w["ff_w2"][128 * ht: 128 * (ht + 1), :])
        w2.append(t)
    for et in range(ET):
        ps = ps_pp.tile([128, 512], F32, tag="pp")
        for ht in range(HID // 128):
            nc.tensor.matmul(
                ps[:], w2[ht][:, 128 * et: 128 * (et + 1)],
                KT[:, 512 * ht: 512 * (ht + 1)],
                start=(ht == 0), stop=(ht == HID // 128 - 1))
        nc.vector.scalar_tensor_tensor(
            A[:, CH * et: CH * (et + 1)], ps[:], vcol("ff_b2", et),
            x2T[:, CH * et: CH * (et + 1)], OP.add, OP.add)
    if d_dbg:
        dma(d_dbg["hT"], KT[:])
    layernorm(A, Bt, "ln3_g", "ln3_b")       # Bt = x3T

    # ================= phase 4: transpose + store =================
    for qt in range(CH // 128):
        for et in range(ET):
            ps = ps_pp.tile([128, 512], F32R, tag="pp")
            nc.tensor.transpose(
                ps[:, 0:128],
                Bt[:, CH * et + 128 * qt: CH * et + 128 * (qt + 1)], ident)
            nc.vector.tensor_copy(
                av[:, CH * qt + 128 * et: CH * qt + 128 * (et + 1)], ps[:, 0:128])
        dma(d_out[128 * qt: 128 * (qt + 1), :], av[:, CH * qt: CH * (qt + 1)])

    ctx.close()


# ---------------------------------------------------------------------------
_PROG = None


def _get_prog():
    global _PROG
    if _PROG is None:
        nc = build_program()
        nc.m = get_hw_module(nc.m)
        _PROG = nc
    return _PROG


def make_in_maps(x, encoder_output, weights):
    """Host-side prep: per-core transposed slices, masks, packed vectors."""
    x = np.asarray(x, np.float32)
    enc = np.asarray(encoder_output, np.float32)
    vec_pack = np.concatenate(
        [np.asarray(weights[n], np.float32).reshape(4, 128).T for n in VEC_NAMES],
        axis=1)
    b1p = np.asarray(weights["ff_b1"], np.float32).reshape(HID // 128, 128).T
    bvrows = np.concatenate([np.asarray(weights["sa_bv"], np.float32),
                             np.asarray(weights["ca_bv"], np.float32)])[None, :]
    shared = {n: np.ascontiguousarray(np.asarray(weights[n], np.float32))
              for n in ("sa_wq", "sa_wk", "sa_wv", "sa_wo",
                        "ca_wq", "ca_wk", "ca_wv", "ca_wo", "ff_w1")}
    shared["ff_w2"] = np.ascontiguousarray(
        np.asarray(weights["ff_w2"], np.float32).astype(ml_dtypes.bfloat16))
    shared["vecs"] = np.ascontiguousarray(vec_pack, np.float32)
    shared["b1p"] = np.ascontiguousarray(b1p, np.float32)
    shared["bvrows"] = np.ascontiguousarray(bvrows, np.float32)
    onesid = np.zeros((128, 129), np.float32)
    onesid[:, 0] = 1.0
    onesid[:, 1:] = np.eye(128, dtype=np.float32)
    shared["onesid"] = onesid

    in_maps = []
    kk = np.arange(128)
    qq = np.arange(512)
    for core in range(NCORES):
        b, c = core // 4, core % 4
        q0 = CH * c
        m = {}
        m["xqT"] = np.ascontiguousarray(x[b, q0:q0 + CH, :].T)
        m["xbT"] = np.ascontiguousarray(x[b].T)
        m["encT"] = np.ascontiguousarray(enc[b].T)
        maskg = np.zeros((NG, 128, 1024), ml_dtypes.bfloat16)
        for g in range(NG):
            for j in range(2):
                kabs = 128 * (2 * g + j) + kk
                maskg[g, :, 512 * j: 512 * (j + 1)] = (
                    kabs[:, None] <= (q0 + qq)[None, :]).astype(ml_dtypes.bfloat16)
        m["maskg"] = maskg
        m.update(shared)
        in_maps.append(m)
    return in_maps


def run(x, encoder_output, weights, trace=False, **kw):
    nc = _get_prog()
    in_maps = make_in_maps(x, encoder_output, weights)
    res = bass_utils.run_bass_kernel_spmd(
        nc, in_maps, core_ids=list(range(NCORES)), trace=trace, **kw)
    out = np.empty((B, N, E), np.float32)
    for core in range(NCORES):
        b, c = core // 4, core % 4
        out[b, CH * c: CH * (c + 1), :] = res.results[core]["out"]
    return out, res


def kernel(**inputs):
    x = inputs.pop("x")
    enc = inputs.pop("encoder_output")
    out, _ = run(x, enc, inputs)
    return out
